# revision 21
# baseline (speedup 1.0000x reference)
"""Trainium2 Bass kernel for nn_Conv2d_20590073217670.

Conv2d: input [32,64,64,64] (NCHW), weight [576,128] (unfold layout:
row = ci*9 + a*3 + b for tap (a,b)), bias [1,128,1,1], stride 1, pad 1.
Output [32,128,64,64].

Strategy: data-parallel over batch — 4 images per NeuronCore, 8 cores.
The host pre-casts the input to bf16 and pre-pads it into
xp[n, c, t, j] = img[n, c, t-1, j-1] (zero border rows/cols); the
device builds two padded [128, 66, *] GEMM layouts:
  xb: parts 0:64  <- DMA xp[n]           (= img[c, r-1, j-1])
      parts 64:128 <- DMA xp[n][:,1:65]  (= img[c, r,   j-1], one row up)
  xc: parts 0:64  <- DVE copy of xb lower
      parts 64:128 <- Act copy of xb lower shifted one col left
                                         (= img[c, r-1, j  ])
Per 8-row output block, 5 matmul passes, each a uniform full [8, 64]
PSUM tile (border taps read the zero padding):
  3x K=128: vertical tap pairs (0,b)+(1,b) from xb        (b = 0,1,2)
  1x K=128: horizontal tap pair (2,0)+(2,1) from xc at +2 rows
  1x K=64 : tap (2,2) from xb lower at +2 rows, col 2
Four blocks are processed pass-major so consecutive matmuls rotate
over 4 PSUM banks and pipeline through the PE array at ~216ns each.

Perf notes (from NTFF traces):
  - The core starts under a 50% utilization throttle and ramps to 100%
    only after ~5us of sustained PE activity; matmuls run at ~427ns
    instead of 216ns until then.  A chain of small warm-up matmuls on
    memset scratch is issued during the (otherwise PE-idle) preamble +
    input-load window so the ramp completes before the real stream.
  - Output is bf16 on device (half the outbound DMA bytes); the host
    casts back to f32.  PSUM eviction fuses bias add + down-cast.
  - Weights are five per-pass [128,128] slabs interleaved with image
    0's input chunks across both HWDGE rings so nothing stalls the
    first passes; image 0 is chunked fine-grained for an early start.
  - The last image's outputs are written per 16 rows alternating
    between the two rings to shorten the drain.
"""
import sys

for _p in ("/opt/trn_rl_repo", "/root/.axon_site/_ro/trn_rl_repo"):
    if _p not in sys.path:
        sys.path.append(_p)

import numpy as np
import ml_dtypes
from contextlib import ExitStack

import concourse.bacc as bacc
import concourse.tile as tile
from concourse import mybir
from concourse.bass_utils import run_bass_kernel_spmd

f32 = mybir.dt.float32
bf16 = mybir.dt.bfloat16

N_CORES = 8
NB = 4  # images per core
N_WARM = 0  # PE warm-up disabled: full-K dummies trip the sustained-power
            # cap (whole stream drops to ~84% rate), K=1 dummies don't feed
            # the utilization-limiter ramp at all


def build_nc():
    nc = bacc.Bacc()
    xp = nc.declare_dram_parameter("xp", [NB, 64, 66, 66], bf16, isOutput=False)
    wph = nc.declare_dram_parameter("wph", [128, 5, 128], bf16, isOutput=False)
    bias = nc.declare_dram_parameter("b", [128, 1], f32, isOutput=False)
    out = nc.declare_dram_parameter("out", [NB, 128, 64, 64], bf16, isOutput=True)

    with tile.TileContext(nc) as tc, ExitStack() as ctx:
        const = ctx.enter_context(tc.tile_pool(name="const", bufs=1))
        xb_pool = ctx.enter_context(tc.tile_pool(name="xb", bufs=3))
        xc_pool = ctx.enter_context(tc.tile_pool(name="xc", bufs=2))
        ob_pool = ctx.enter_context(tc.tile_pool(name="ob", bufs=2))
        ps_pool = ctx.enter_context(tc.tile_pool(name="ps", bufs=1, space="PSUM"))

        if N_WARM:
            # PE warm-up: small matmuls on memset scratch rotating over this
            # generation's 4 PSUM banks, holding PE activity high while the
            # hardware utilization limiter ramps 50% -> 100%.
            wsrc = const.tile([128, 4, 64], bf16)
            nc.vector.memset(wsrc[:], 0.0)
            # reuse the P0..P3 slot names so the pool footprint stays 8 banks
            D = [ps_pool.tile([128, 8, 64], f32, name=f"P{i}") for i in range(4)]
            for i in range(N_WARM):
                nc.tensor.matmul(
                    D[i % 4][:, 0:4, :], wsrc[0:1, 0:2, :], wsrc[0:1, :, :],
                    start=True, stop=True,
                )

        # ---- weights, pre-arranged bf16 on the host as 5 per-pass slabs
        # wph[p] is the [128,128] stationary for pass p:
        #   p=0..2: lower = tap (0,p), upper = tap (1,p)
        #   p=3:    lower = tap (2,0), upper = tap (2,1)
        #   p=4:    lower = tap (2,2), upper = zeros (pass runs K=64)
        wt = const.tile([128, 5, 128], bf16)
        bt = const.tile([128, 1], f32)

        def emit_image_dmas(n):
            """DMA xp[n] into a fresh xb tile (lower + row-shifted upper).
            Image 0 is chunked: lower rides the sync ring behind the single
            weight load, upper gets the scalar ring to itself, so the first
            matmuls start as early as both rings allow.  The bias (tiny but
            128 four-byte packets) goes last - it is only needed by the
            first eviction."""
            xb = xb_pool.tile([128, 66, 66], bf16)
            if n == 0:
                nc.sync.dma_start(out=wt[:, 0:2, :], in_=wph[:, 0:2, :])
                nc.sync.dma_start(out=xb[0:64, 0:10, :], in_=xp[0][:, 0:10, :])
                nc.sync.dma_start(out=xb[0:64, 10:35, :], in_=xp[0][:, 10:35, :])
                nc.sync.dma_start(out=wt[:, 2:5, :], in_=wph[:, 2:5, :])
                nc.sync.dma_start(out=xb[0:64, 35:66, :], in_=xp[0][:, 35:66, :])
                nc.scalar.dma_start(out=xb[64:128, 0:9, :], in_=xp[0][:, 1:10, :])
                nc.scalar.dma_start(out=xb[64:128, 9:34, :], in_=xp[0][:, 10:35, :])
                nc.scalar.dma_start(out=xb[64:128, 34:64, :], in_=xp[0][:, 35:65, :])
                nc.scalar.dma_start(out=bt[:], in_=bias[:])
            else:
                nc.sync.dma_start(out=xb[0:64, :, :], in_=xp[n][:, :, :])
                nc.sync.dma_start(out=xb[64:128, 0:64, :], in_=xp[n][:, 1:65, :])
            return xb

        def emit_image_copies(n, xb):
            """Derive xc from xb: lower = xb lower; upper = one col left
            (img[c, r-1, j]).  Only rows 2:66 / cols 0:65 are ever read."""
            xc = xc_pool.tile([128, 66, 65], bf16)
            if n == 0:
                nc.vector.tensor_copy(xc[0:64, 2:35, :], xb[0:64, 2:35, 0:65])
                nc.vector.tensor_copy(xc[64:128, 2:35, :], xb[0:64, 2:35, 1:66])
                nc.scalar.copy(xc[0:64, 35:66, :], xb[0:64, 35:66, 0:65])
                nc.scalar.copy(xc[64:128, 35:66, :], xb[0:64, 35:66, 1:66])
            else:
                nc.vector.tensor_copy(xc[0:64, 2:66, :], xb[0:64, 2:66, 0:65])
                nc.scalar.copy(xc[64:128, 2:66, :], xb[0:64, 2:66, 1:66])
            return xc

        xb_cur = emit_image_dmas(0)
        tiles = (xb_cur, emit_image_copies(0, xb_cur))

        def emit_group(n, xb, xc, osb, Ps, ys, last):
            """Pass-major matmuls over len(Ps) PSUM banks, then eviction
            (fused bias add + f32->bf16 down-cast) split across DVE/Act,
            then the output DMA for the covered rows."""
            for p in range(5):
                st, sp = (p == 0), (p == 4)
                for P, y0 in zip(Ps, ys):
                    if p < 3:
                        nc.tensor.matmul(
                            P[:, :, :], wt[:, p, :],
                            xb[:, y0:y0 + 8, p:p + 64],
                            start=st, stop=sp,
                        )
                    elif p == 3:
                        nc.tensor.matmul(
                            P[:, :, :], wt[:, 3, :],
                            xc[:, y0 + 2:y0 + 10, 0:64],
                            start=st, stop=sp,
                        )
                    else:
                        nc.tensor.matmul(
                            P[:, :, :], wt[0:64, 4, :],
                            xb[0:64, y0 + 2:y0 + 10, 2:66],
                            start=st, stop=sp,
                        )
            for q, (P, y0) in enumerate(zip(Ps, ys)):
                if not last and len(ys) == 8 and q == 4:
                    # first half's rows are all evicted: stream them out now
                    nc.scalar.dma_start(
                        out=out[n][:, 0:32, :], in_=osb[:, 0:32, :])
                if last:
                    # tail: split each eviction row-wise across both
                    # engines; rows 0:32 stream out after the 4th eviction,
                    # the rest as two parallel 16-row pieces at the end
                    nc.vector.tensor_scalar_add(
                        osb[:, y0:y0 + 4, :], P[:, 0:4, :], bt[:])
                    nc.scalar.add(
                        osb[:, y0 + 4:y0 + 8, :], P[:, 4:8, :], bt[:])
                    if q == 3:
                        nc.sync.dma_start(
                            out=out[n][:, 0:32, :], in_=osb[:, 0:32, :])
                    elif q == 7:
                        nc.scalar.dma_start(
                            out=out[n][:, 32:48, :], in_=osb[:, 32:48, :])
                        nc.sync.dma_start(
                            out=out[n][:, 48:64, :], in_=osb[:, 48:64, :])
                elif q % 2 == 1:
                    nc.scalar.add(osb[:, y0:y0 + 8, :], P[:, :, :], bt[:])
                else:
                    nc.vector.tensor_scalar_add(
                        osb[:, y0:y0 + 8, :], P[:, :, :], bt[:])
            if not last and len(ys) == 4:
                y0, y1 = ys[0], ys[-1] + 8
                nc.scalar.dma_start(
                    out=out[n][:, y0:y1, :], in_=osb[:, y0:y1, :])
            elif not last:
                nc.scalar.dma_start(
                    out=out[n][:, 32:64, :], in_=osb[:, 32:64, :])

        for n in range(NB):
            xb, xc = tiles
            if n + 1 < NB:
                # issue next image's input DMAs now: they get maximum lead
                # on the sync ring (which carries only input in steady state)
                xb_next = emit_image_dmas(n + 1)

            osb = ob_pool.tile([128, 64, 64], bf16)
            P0 = ps_pool.tile([128, 8, 64], f32, name="P0", bufs=1)
            P1 = ps_pool.tile([128, 8, 64], f32, name="P1", bufs=1)
            P2 = ps_pool.tile([128, 8, 64], f32, name="P2", bufs=1)
            P3 = ps_pool.tile([128, 8, 64], f32, name="P3", bufs=1)
            P4 = ps_pool.tile([128, 8, 64], f32, name="P4", bufs=1)
            P5 = ps_pool.tile([128, 8, 64], f32, name="P5", bufs=1)
            P6 = ps_pool.tile([128, 8, 64], f32, name="P6", bufs=1)
            P7 = ps_pool.tile([128, 8, 64], f32, name="P7", bufs=1)
            banks = (P0, P1, P2, P3, P4, P5, P6, P7)
            last = n == NB - 1
            if n == 0:
                # image 0 as two 4-bank groups so the first matmuls only
                # gate on the first input chunks
                emit_group(0, xb, xc, osb, banks[0:4], [0, 8, 16, 24], False)
                emit_group(0, xb, xc, osb, banks[4:8], [32, 40, 48, 56], False)
            else:
                # one pass-major group over all 8 banks: weights reused 8x,
                # half the group-boundary hiccups
                emit_group(n, xb, xc, osb, banks,
                           [q * 8 for q in range(8)], last)
            if n + 1 < NB:
                # next image's xc copies run on DVE/Act behind this image's
                # evictions
                tiles = (xb_next, emit_image_copies(n + 1, xb_next))

    nc.finalize()
    return nc


_NC = None


def _get_nc():
    global _NC
    if _NC is None:
        _NC = build_nc()
    return _NC


def host_prep(inputs):
    x = np.asarray(inputs["input"], dtype=np.float32)
    w = np.asarray(inputs["weight"], dtype=np.float32)
    b = np.ascontiguousarray(
        np.asarray(inputs["bias"], dtype=np.float32).reshape(128, 1))
    # host-side bf16 cast + zero padding: xp[n, c, t, j] = x[n, c, t-1, j-1]
    N = x.shape[0]
    xp = np.zeros((N, 64, 66, 66), dtype=ml_dtypes.bfloat16)
    xp[:, :, 1:65, 1:65] = x.astype(ml_dtypes.bfloat16)
    # per-pass weight slabs (see build_nc)
    w3 = w.reshape(64, 9, 128).astype(ml_dtypes.bfloat16)
    wph = np.zeros((5, 128, 128), dtype=ml_dtypes.bfloat16)
    for p in range(3):
        wph[p, 0:64] = w3[:, p]
        wph[p, 64:128] = w3[:, 3 + p]
    wph[3, 0:64] = w3[:, 6]
    wph[3, 64:128] = w3[:, 7]
    wph[4, 0:64] = w3[:, 8]
    # device loads the weights as one [128, 5*128] DMA (contiguous per
    # partition), so transpose to partition-major
    wph = np.ascontiguousarray(wph.transpose(1, 0, 2))
    return xp, wph, b


def kernel(**inputs) -> np.ndarray:
    xp, wph, b = host_prep(inputs)
    nc = _get_nc()
    in_maps = [
        {"xp": xp[c * NB:(c + 1) * NB], "wph": wph, "b": b}
        for c in range(N_CORES)
    ]
    res = run_bass_kernel_spmd(nc, in_maps, list(range(N_CORES)))
    return np.concatenate(
        [np.asarray(r["out"], dtype=np.float32) for r in res.results], axis=0)


# revision 22
# speedup vs baseline: 33229.5940x; 33229.5940x over previous
"""Trainium2 Bass kernel for nn_Conv2d_20590073217670.

Conv2d: input [32,64,64,64] (NCHW), weight [576,128] (unfold layout:
row = ci*9 + a*3 + b for tap (a,b)), bias [1,128,1,1], stride 1, pad 1.
Output [32,128,64,64].

Strategy: data-parallel over batch — 4 images per NeuronCore, 8 cores.
The host pre-casts the input to bf16 and pre-pads it into
xp[n, c, t, j] = img[n, c, t-1, j-1] (zero border rows/cols); the
device builds two padded [128, 66, *] GEMM layouts:
  xb: parts 0:64  <- DMA xp[n]           (= img[c, r-1, j-1])
      parts 64:128 <- DMA xp[n][:,1:65]  (= img[c, r,   j-1], one row up)
  xc: parts 0:64  <- DVE copy of xb lower
      parts 64:128 <- Act copy of xb lower shifted one col left
                                         (= img[c, r-1, j  ])
Per 8-row output block, 5 matmul passes, each a uniform full [8, 64]
PSUM tile (border taps read the zero padding):
  3x K=128: vertical tap pairs (0,b)+(1,b) from xb        (b = 0,1,2)
  1x K=128: horizontal tap pair (2,0)+(2,1) from xc at +2 rows
  1x K=64 : tap (2,2) from xb lower at +2 rows, col 2
Blocks are processed pass-major so consecutive matmuls rotate over the
PSUM banks and pipeline through the PE array at ~216ns each: image 0
as two 4-bank groups (so the first matmuls only gate on the first
input chunks), images 1-3 as one 8-bank group each (weights reused 8x,
half the group-boundary hiccups).

Perf notes (from NTFF traces):
  - Output is bf16 on device (half the outbound DMA bytes); the host
    casts back to f32.  PSUM eviction fuses bias add + down-cast and
    alternates DVE/Act.
  - The weight load is one [128, 5*128] DMA (contiguous per partition;
    per-pass [5,128,128] slabs generate 5x128 tiny packets instead).
    The [128,1] f32 bias is 128 four-byte packets, so it rides last.
  - Image 0's input is chunked: lower behind the weights on the sync
    HWDGE ring, upper on the scalar ring, first chunks ~10 rows so the
    first matmul starts ~2.5us after the rings come up.
  - The core modulates PE throughput (50% duty at kernel start ramping
    to 100% after ~4-7us of sustained streaming; ~84% sustained cap
    when the chip is hot from recent runs).  Warm-up dummy matmuls do
    not help: full-K dummies spend the same activity budget, K=1
    dummies do not feed the ramp.  Run-to-run variance from this is
    +/-2us on a ~58us kernel, more if the previous run just finished.
  - The last image's evictions are split row-wise across DVE+Act and
    its output leaves as rows 0:32 (after the 4th eviction) plus two
    parallel 16-row pieces on both rings, shortening the drain.
"""
import sys

for _p in ("/opt/trn_rl_repo", "/root/.axon_site/_ro/trn_rl_repo"):
    if _p not in sys.path:
        sys.path.append(_p)

import numpy as np
import ml_dtypes
from contextlib import ExitStack

import concourse.bacc as bacc
import concourse.tile as tile
from concourse import mybir
from concourse.bass_utils import run_bass_kernel_spmd

f32 = mybir.dt.float32
bf16 = mybir.dt.bfloat16

N_CORES = 8
NB = 4  # images per core
N_WARM = 0  # PE warm-up disabled: full-K dummies trip the sustained-power
            # cap (whole stream drops to ~84% rate), K=1 dummies don't feed
            # the utilization-limiter ramp at all


def build_nc():
    nc = bacc.Bacc()
    xp = nc.declare_dram_parameter("xp", [NB, 64, 66, 66], bf16, isOutput=False)
    wph = nc.declare_dram_parameter("wph", [128, 5, 128], bf16, isOutput=False)
    bias = nc.declare_dram_parameter("b", [128, 1], f32, isOutput=False)
    out = nc.declare_dram_parameter("out", [NB, 128, 64, 64], bf16, isOutput=True)

    with tile.TileContext(nc) as tc, ExitStack() as ctx:
        const = ctx.enter_context(tc.tile_pool(name="const", bufs=1))
        xb_pool = ctx.enter_context(tc.tile_pool(name="xb", bufs=3))
        xc_pool = ctx.enter_context(tc.tile_pool(name="xc", bufs=2))
        ob_pool = ctx.enter_context(tc.tile_pool(name="ob", bufs=2))
        ps_pool = ctx.enter_context(tc.tile_pool(name="ps", bufs=1, space="PSUM"))

        if N_WARM:
            # PE warm-up: small matmuls on memset scratch rotating over this
            # generation's 4 PSUM banks, holding PE activity high while the
            # hardware utilization limiter ramps 50% -> 100%.
            wsrc = const.tile([128, 4, 64], bf16)
            nc.vector.memset(wsrc[:], 0.0)
            # reuse the P0..P3 slot names so the pool footprint stays 8 banks
            D = [ps_pool.tile([128, 8, 64], f32, name=f"P{i}") for i in range(4)]
            for i in range(N_WARM):
                nc.tensor.matmul(
                    D[i % 4][:, 0:4, :], wsrc[0:1, 0:2, :], wsrc[0:1, :, :],
                    start=True, stop=True,
                )

        # ---- weights, pre-arranged bf16 on the host as 5 per-pass slabs
        # wph[p] is the [128,128] stationary for pass p:
        #   p=0..2: lower = tap (0,p), upper = tap (1,p)
        #   p=3:    lower = tap (2,0), upper = tap (2,1)
        #   p=4:    lower = tap (2,2), upper = zeros (pass runs K=64)
        wt = const.tile([128, 5, 128], bf16)
        bt = const.tile([128, 1], f32)

        def emit_image_dmas(n):
            """DMA xp[n] into a fresh xb tile (lower + row-shifted upper).
            Image 0 is chunked: lower rides the sync ring behind the single
            weight load, upper gets the scalar ring to itself, so the first
            matmuls start as early as both rings allow.  The bias (tiny but
            128 four-byte packets) goes last - it is only needed by the
            first eviction."""
            xb = xb_pool.tile([128, 66, 66], bf16)
            if n == 0:
                nc.sync.dma_start(out=wt[:, 0:2, :], in_=wph[:, 0:2, :])
                nc.sync.dma_start(out=xb[0:64, 0:10, :], in_=xp[0][:, 0:10, :])
                nc.sync.dma_start(out=xb[0:64, 10:35, :], in_=xp[0][:, 10:35, :])
                nc.sync.dma_start(out=wt[:, 2:5, :], in_=wph[:, 2:5, :])
                nc.sync.dma_start(out=xb[0:64, 35:66, :], in_=xp[0][:, 35:66, :])
                nc.scalar.dma_start(out=xb[64:128, 0:9, :], in_=xp[0][:, 1:10, :])
                nc.scalar.dma_start(out=xb[64:128, 9:34, :], in_=xp[0][:, 10:35, :])
                nc.scalar.dma_start(out=xb[64:128, 34:64, :], in_=xp[0][:, 35:65, :])
                nc.scalar.dma_start(out=bt[:], in_=bias[:])
            else:
                nc.sync.dma_start(out=xb[0:64, :, :], in_=xp[n][:, :, :])
                nc.sync.dma_start(out=xb[64:128, 0:64, :], in_=xp[n][:, 1:65, :])
            return xb

        def emit_image_copies(n, xb):
            """Derive xc from xb: lower = xb lower; upper = one col left
            (img[c, r-1, j]).  Only rows 2:66 / cols 0:65 are ever read."""
            xc = xc_pool.tile([128, 66, 65], bf16)
            if n == 0:
                nc.vector.tensor_copy(xc[0:64, 2:35, :], xb[0:64, 2:35, 0:65])
                nc.vector.tensor_copy(xc[64:128, 2:35, :], xb[0:64, 2:35, 1:66])
                nc.scalar.copy(xc[0:64, 35:66, :], xb[0:64, 35:66, 0:65])
                nc.scalar.copy(xc[64:128, 35:66, :], xb[0:64, 35:66, 1:66])
            else:
                nc.vector.tensor_copy(xc[0:64, 2:66, :], xb[0:64, 2:66, 0:65])
                nc.scalar.copy(xc[64:128, 2:66, :], xb[0:64, 2:66, 1:66])
            return xc

        xb_cur = emit_image_dmas(0)
        tiles = (xb_cur, emit_image_copies(0, xb_cur))

        def emit_group(n, xb, xc, osb, Ps, ys, last):
            """Pass-major matmuls over len(Ps) PSUM banks, then eviction
            (fused bias add + f32->bf16 down-cast) split across DVE/Act,
            then the output DMA for the covered rows."""
            for p in range(5):
                st, sp = (p == 0), (p == 4)
                for P, y0 in zip(Ps, ys):
                    if p < 3:
                        nc.tensor.matmul(
                            P[:, :, :], wt[:, p, :],
                            xb[:, y0:y0 + 8, p:p + 64],
                            start=st, stop=sp,
                        )
                    elif p == 3:
                        nc.tensor.matmul(
                            P[:, :, :], wt[:, 3, :],
                            xc[:, y0 + 2:y0 + 10, 0:64],
                            start=st, stop=sp,
                        )
                    else:
                        nc.tensor.matmul(
                            P[:, :, :], wt[0:64, 4, :],
                            xb[0:64, y0 + 2:y0 + 10, 2:66],
                            start=st, stop=sp,
                        )
            for q, (P, y0) in enumerate(zip(Ps, ys)):
                if not last and len(ys) == 8 and q == 4:
                    # first half's rows are all evicted: stream them out now
                    nc.scalar.dma_start(
                        out=out[n][:, 0:32, :], in_=osb[:, 0:32, :])
                if last:
                    # tail: split each eviction row-wise across both
                    # engines; rows 0:32 stream out after the 4th eviction,
                    # the rest as two parallel 16-row pieces at the end
                    nc.vector.tensor_scalar_add(
                        osb[:, y0:y0 + 4, :], P[:, 0:4, :], bt[:])
                    nc.scalar.add(
                        osb[:, y0 + 4:y0 + 8, :], P[:, 4:8, :], bt[:])
                    if q == 3:
                        nc.sync.dma_start(
                            out=out[n][:, 0:32, :], in_=osb[:, 0:32, :])
                    elif q == 7:
                        nc.scalar.dma_start(
                            out=out[n][:, 32:48, :], in_=osb[:, 32:48, :])
                        nc.sync.dma_start(
                            out=out[n][:, 48:64, :], in_=osb[:, 48:64, :])
                elif q % 2 == 1:
                    nc.scalar.add(osb[:, y0:y0 + 8, :], P[:, :, :], bt[:])
                else:
                    nc.vector.tensor_scalar_add(
                        osb[:, y0:y0 + 8, :], P[:, :, :], bt[:])
            if not last and len(ys) == 4:
                y0, y1 = ys[0], ys[-1] + 8
                nc.scalar.dma_start(
                    out=out[n][:, y0:y1, :], in_=osb[:, y0:y1, :])
            elif not last:
                nc.scalar.dma_start(
                    out=out[n][:, 32:64, :], in_=osb[:, 32:64, :])

        for n in range(NB):
            xb, xc = tiles
            if n + 1 < NB:
                # issue next image's input DMAs now: they get maximum lead
                # on the sync ring (which carries only input in steady state)
                xb_next = emit_image_dmas(n + 1)

            osb = ob_pool.tile([128, 64, 64], bf16)
            P0 = ps_pool.tile([128, 8, 64], f32, name="P0", bufs=1)
            P1 = ps_pool.tile([128, 8, 64], f32, name="P1", bufs=1)
            P2 = ps_pool.tile([128, 8, 64], f32, name="P2", bufs=1)
            P3 = ps_pool.tile([128, 8, 64], f32, name="P3", bufs=1)
            P4 = ps_pool.tile([128, 8, 64], f32, name="P4", bufs=1)
            P5 = ps_pool.tile([128, 8, 64], f32, name="P5", bufs=1)
            P6 = ps_pool.tile([128, 8, 64], f32, name="P6", bufs=1)
            P7 = ps_pool.tile([128, 8, 64], f32, name="P7", bufs=1)
            banks = (P0, P1, P2, P3, P4, P5, P6, P7)
            last = n == NB - 1
            if n == 0:
                # image 0 as two 4-bank groups so the first matmuls only
                # gate on the first input chunks
                emit_group(0, xb, xc, osb, banks[0:4], [0, 8, 16, 24], False)
                emit_group(0, xb, xc, osb, banks[4:8], [32, 40, 48, 56], False)
            else:
                # one pass-major group over all 8 banks: weights reused 8x,
                # half the group-boundary hiccups
                emit_group(n, xb, xc, osb, banks,
                           [q * 8 for q in range(8)], last)
            if n + 1 < NB:
                # next image's xc copies run on DVE/Act behind this image's
                # evictions
                tiles = (xb_next, emit_image_copies(n + 1, xb_next))

    nc.finalize()
    return nc


_NC = None


def _get_nc():
    global _NC
    if _NC is None:
        _NC = build_nc()
    return _NC


def host_prep(inputs):
    x = np.asarray(inputs["input"], dtype=np.float32)
    w = np.asarray(inputs["weight"], dtype=np.float32)
    b = np.ascontiguousarray(
        np.asarray(inputs["bias"], dtype=np.float32).reshape(128, 1))
    # host-side bf16 cast + zero padding: xp[n, c, t, j] = x[n, c, t-1, j-1]
    N = x.shape[0]
    xp = np.zeros((N, 64, 66, 66), dtype=ml_dtypes.bfloat16)
    xp[:, :, 1:65, 1:65] = x.astype(ml_dtypes.bfloat16)
    # per-pass weight slabs (see build_nc)
    w3 = w.reshape(64, 9, 128).astype(ml_dtypes.bfloat16)
    wph = np.zeros((5, 128, 128), dtype=ml_dtypes.bfloat16)
    for p in range(3):
        wph[p, 0:64] = w3[:, p]
        wph[p, 64:128] = w3[:, 3 + p]
    wph[3, 0:64] = w3[:, 6]
    wph[3, 64:128] = w3[:, 7]
    wph[4, 0:64] = w3[:, 8]
    # device loads the weights as one [128, 5*128] DMA (contiguous per
    # partition), so transpose to partition-major
    wph = np.ascontiguousarray(wph.transpose(1, 0, 2))
    return xp, wph, b


def kernel(**inputs) -> np.ndarray:
    xp, wph, b = host_prep(inputs)
    nc = _get_nc()
    in_maps = [
        {"xp": xp[c * NB:(c + 1) * NB], "wph": wph, "b": b}
        for c in range(N_CORES)
    ]
    res = run_bass_kernel_spmd(nc, in_maps, list(range(N_CORES)))
    return np.concatenate(
        [np.asarray(r["out"], dtype=np.float32) for r in res.results], axis=0)
